# revision 38
# baseline (speedup 1.0000x reference)
"""Trainium2 Bass kernel for a GQA attention layer (S=2048, HID=4096, H=32, KV=8, D=128).

Sharding: tensor-parallel over heads across 8 NeuronCores. Core c computes
q heads [4c, 4c+4) and kv head c end-to-end (QKV proj -> RMSNorm -> RoPE ->
causal flash-style attention -> partial o_proj). Each core returns a partial
[S, HID] bf16 o_proj output (w_o column-sharded); the host sums the 8 partials.

Device layout notes:
- All projections run with the feature dim on PSUM partitions: qkv_out[f, s] =
  (w_qkvT tile).T @ hidden_T tile, so q/k arrive as [D, S] (head dim on
  partitions), which is exactly the layout the scores matmul needs
  (contraction over D).
- Scores are computed transposed: scoresT[k, q] via lhsT=kT tile [D, Sk-128],
  rhs=qT [D, Sq-512]. The softmax denominator is ones[128,128].T @ exp(scoresT),
  which also broadcasts the per-q sum across all 128 partitions so the
  normalization multiply needs no cross-partition traffic. No max-subtraction:
  scores are O(5) for RMS-normed q/k, so exp is safe in fp32.
- Causal masking at 128-column granularity: blocks above the diagonal are
  skipped outright; the 4 diagonal-band blocks per (head, sq-chunk) run
  partial-free-dim matmuls (scores AND ctx/den accumulation restricted to
  columns >= r*128) plus one [128,128] triangle mask multiply after exp.
  Zero-column overhead vs the causal ideal at this granularity.
- Single fused pipeline: chunk c's QKV matmuls, chunk c-1's attention and
  chunk c-2's o_proj are emitted interleaved, so the PE queue never drains
  across phase boundaries (HAM stays warm) and ACT/DVE epilogues hide under
  matmuls of neighboring stages.
- Matmul operands are bf16 (fp32 PSUM accumulation); the rmsnorm/rope/softmax
  normalization chain stays fp32. fp8 was measured and rejected: e4m3's ~2.6%
  per-element quantization noise does NOT average down in dot products, giving
  2.3-6.8e-2 output rel-err per fp8 stage (vs the 2e-2 budget).
"""

import numpy as np

import concourse.bass as bass
from concourse import bacc
import concourse.tile as tile
from concourse import mybir
from concourse.bass_utils import run_bass_kernel_spmd
from concourse.masks import make_identity

F32 = mybir.dt.float32
F16 = mybir.dt.float16
BF16 = mybir.dt.bfloat16

S = 2048
HID = 4096
H = 32
KV = 8
D = 128
QH = H // KV  # 4 q heads per kv head; with 8 cores -> 1 kv head per core
NCORES = 8
EPS = 1e-6
THETA = 10000.0
HALF = D // 2

ST = 512  # seq chunk (matmul free dim)
N_ST = S // ST  # 4
KT = HID // D  # 32 contraction tiles for qkv
NB = QH + 2  # 6 feature blocks per core: q0..q3, k, v
SK = S // D  # 16 key blocks of 128
SCALE = float(D) ** -0.5
RB = ST // D  # 4 row-tiles / diagonal bands per chunk

# feature-block waves per chunk, in emission order; v last. rmsnorm uses
# rstd = exp(-0.5*ln(mean+eps)) with ALL five ln calls batched (one
# natural-log table load, hidden under the q23/v waves) followed by the
# rstd exps, which share the SAME activation-table set as the attention
# exps -- so no table switch can ever land on the attention critical path,
# and the q2/q3 finishing work is free to lag into the attention window.
WAVES = [(0, 1), (4,), (2, 3), (5,)]  # 4 = k, 5 = v


def build_bass():
    nc = bacc.Bacc(
        "TRN2", target_bir_lowering=False, debug=False, num_devices=NCORES
    )
    hid_d = nc.dram_tensor("hidden_t", [HID, S], BF16, kind="ExternalInput").ap()
    wqkv_d = nc.dram_tensor("w_qkvT", [HID, NB * D], BF16, kind="ExternalInput").ap()
    wo_d = nc.dram_tensor("w_oT", [QH * D, HID], BF16, kind="ExternalInput").ap()
    cosq_d = nc.dram_tensor("cosq", [D, S], F16, kind="ExternalInput").ap()
    sinq_d = nc.dram_tensor("sinq", [D, S], F16, kind="ExternalInput").ap()
    cosk_d = nc.dram_tensor("cosk", [D, S], F16, kind="ExternalInput").ap()
    sink_d = nc.dram_tensor("sink", [D, S], F16, kind="ExternalInput").ap()
    mask_d = nc.dram_tensor("mask_tri", [D, D], BF16, kind="ExternalInput").ap()
    out_d = nc.dram_tensor("out_partial", [S, HID], BF16, kind="ExternalOutput").ap()

    with tile.TileContext(nc) as tc:
        build_kernel(
            nc, tc, hid_d, wqkv_d, wo_d,
            (cosq_d, sinq_d, cosk_d, sink_d), mask_d, out_d,
        )
    nc.finalize()
    return nc


def build_kernel(nc, tc, hid_d, wqkv_d, wo_d, tables_d, mask_d, out_d):
    from contextlib import ExitStack

    cosq_d, sinq_d, cosk_d, sink_d = tables_d

    with ExitStack() as ctxs:
        # ---- persistent SBUF ----
        qk_pool = ctxs.enter_context(tc.tile_pool(name="qk", bufs=1))
        const_pool = ctxs.enter_context(tc.tile_pool(name="const", bufs=1))
        w_pool = ctxs.enter_context(tc.tile_pool(name="wqkv", bufs=1))
        wo_pool = ctxs.enter_context(tc.tile_pool(name="wo", bufs=1))
        ctx_pool = ctxs.enter_context(tc.tile_pool(name="ctx", bufs=1))
        hid_pool = ctxs.enter_context(tc.tile_pool(name="hid", bufs=9))
        tbl_pool = ctxs.enter_context(tc.tile_pool(name="tbl", bufs=2))
        tmp_pool = ctxs.enter_context(tc.tile_pool(name="p1tmp", bufs=1))
        drain_pool = ctxs.enter_context(tc.tile_pool(name="drain", bufs=4))
        e_pool = ctxs.enter_context(tc.tile_pool(name="expp", bufs=6))
        a_tmp = ctxs.enter_context(tc.tile_pool(name="p2tmp", bufs=3))
        o_pool = ctxs.enter_context(tc.tile_pool(name="outst", bufs=2))
        # all 5 ln outputs of a chunk stay live until their rstd exps run
        # (which happens after ALL lns -- fewer slots would deadlock the
        # in-order ACT queue)
        lnm_pool = ctxs.enter_context(tc.tile_pool(name="lnm", bufs=6))
        # PSUM budget (bufs is per tile NAME): qkv wave slots lo/hi x2 = 4
        # banks; one shared scratch name (msum/v-transpose/scores/o_proj)
        # x2 = 2; ctx + den x1 each = 2  -> 8 banks total
        qkv_ps_pool = ctxs.enter_context(
            tc.tile_pool(name="qkv_ps", bufs=2, space="PSUM")
        )
        scr_ps_pool = ctxs.enter_context(
            tc.tile_pool(name="scr_ps", bufs=2, space="PSUM")
        )
        cd_ps_pool = ctxs.enter_context(
            tc.tile_pool(name="cd_ps", bufs=1, space="PSUM")
        )

        def scr_tile():
            return scr_ps_pool.tile([D, ST], F32, name="scr_ps")

        qkT = [qk_pool.tile([D, S], BF16, name=f"qkT_{m}") for m in range(QH + 1)]
        v_sb = qk_pool.tile([D, SK, D], BF16, name="v_sb")
        ctx_sb = [qk_pool.tile([D, S], BF16, name=f"ctx_{h}") for h in range(QH)]
        wqkv_sb = w_pool.tile([D, KT, NB * D], BF16, name="wqkv_sb")
        wo_sb = wo_pool.tile([D, QH, HID], BF16, name="wo_sb")
        mask_sb = const_pool.tile([D, D], BF16, name="mask_sb")

        identity = const_pool.tile([D, D], F32, name="identity")
        make_identity(nc, identity[:])
        ones_f32 = const_pool.tile([D, D], F32, name="ones_f32")
        nc.vector.memset(ones_f32[:], 1.0)
        ones_sb = const_pool.tile([D, D], BF16, name="ones_sb")
        nc.vector.tensor_copy(ones_sb[:], ones_f32[:])
        eps_sb = const_pool.tile([D, 1], F32, name="eps_sb")
        nc.vector.memset(eps_sb[:], EPS)

        # triangle mask on the gpsimd DMA queue (the sync queue carries the
        # streaming hidden/table loads). The (big) w_o prefetch is emitted
        # after chunk 0's first wave so it can't delay the wqkv slices the
        # very first matmul needs.
        nc.gpsimd.dma_start(mask_sb[:], mask_d)

        hid_tiles = [[None] * KT for _ in range(N_ST)]
        tbl_tiles = [None] * N_ST

        def load_tables(c):
            # rope tables ride the (idle) gpsimd DMA queue, off the sync
            # queue's critical startup path
            tbls = []
            for i, td in enumerate((cosq_d, sinq_d, cosk_d, sink_d)):
                t = tbl_pool.tile([D, ST], F16, name=f"tbl_{i}")
                nc.gpsimd.dma_start(t[:], td[:, bass.ts(c, ST)])
                tbls.append(t)
            tbl_tiles[c] = tbls

        # hidden loads are batched 4 kt-tiles per DMA: each DIRECT2D costs
        # ~600ns of issue time on the sync queue regardless of size, and
        # 128 separate loads would choke the queue
        def load_hid_batch(c, tb):
            t = hid_pool.tile([D, 4, ST], BF16, name="hid_t")
            nc.sync.dma_start(
                t[:],
                hid_d[bass.ts(tb, 4 * D), bass.ts(c, ST)].rearrange(
                    "(t p) n -> p t n", p=D
                ),
            )
            for i in range(4):
                hid_tiles[c][4 * tb + i] = t[:, i, :]

        def load_hid(c):
            for tb in range(KT // 4):
                load_hid_batch(c, tb)

        def qkv_wave(c, wave, do_loads, mid_kt=None, mid_cb=None):
            qkv_ps = {
                m: qkv_ps_pool.tile(
                    [D, ST], F32, name=f"qkv_ps_{'lo' if i == 0 else 'hi'}"
                )
                for i, m in enumerate(wave)
            }
            for kt in range(KT):
                if kt == mid_kt:
                    mid_cb()
                if do_loads:
                    # interleave weight + hidden loads on the (fast HWDGE)
                    # sync queue so the PE starts immediately: kt 0..3 as
                    # singles (first matmul waits only two small DMAs), the
                    # rest 4-batched to bound queue-issue overhead
                    if kt < 4:
                        nc.sync.dma_start(
                            wqkv_sb[:, kt, :], wqkv_d[bass.ts(kt, D), :]
                        )
                        if kt == 0:
                            t0 = hid_pool.tile([D, 4, ST], BF16, name="hid_t")
                        nc.sync.dma_start(
                            t0[:, kt, :], hid_d[bass.ts(kt, D), bass.ts(c, ST)]
                        )
                        hid_tiles[c][kt] = t0[:, kt, :]
                    elif kt % 4 == 0:
                        tb = kt // 4
                        nc.sync.dma_start(
                            wqkv_sb[:, bass.ts(tb, 4), :],
                            wqkv_d[bass.ts(tb, 4 * D), :].rearrange(
                                "(t p) n -> p t n", p=D
                            ),
                        )
                        load_hid_batch(c, tb)
                t = hid_tiles[c][kt]
                for m in wave:
                    nc.tensor.matmul(
                        qkv_ps[m][:],
                        wqkv_sb[:, kt, bass.ts(m, D)],
                        t,
                        start=(kt == 0),
                        stop=(kt == KT - 1),
                    )
            return qkv_ps

        def epilogue_drains(c, blocks, qkv_ps):
            # PSUM drains + squared copies (ACT + DVE, no PE)
            state = {}
            for m in blocks:
                raw = drain_pool.tile([D, ST], F32, name="raw")
                nc.scalar.copy(raw[:], qkv_ps[m][:])
                if m == QH + 1:
                    state[m] = (raw, None)
                    continue
                sq = tmp_pool.tile([D, ST], BF16, name="sq")
                nc.vector.tensor_mul(sq[:], raw[:], raw[:])
                state[m] = (raw, sq)
            return (c, blocks, state)

        def epilogue_msums(st8):
            # (PE) mean-square matmuls; emitted where the drains are known
            # to have completed so the PE never stalls on them
            c, blocks, state = st8
            for m in blocks:
                raw, sq = state[m]
                if m == QH + 1:
                    continue
                msum = scr_tile()
                nc.tensor.matmul(msum[:], ones_sb[:], sq[:], start=True, stop=True)
                state[m] = (raw, msum)
            return (c, blocks, state)

        def epilogue_ln(st8):
            # ln(mean + eps) for each q/k block; all lns of a chunk are
            # emitted back-to-back so the natural-log table loads once
            c, blocks, state = st8
            for m in blocks:
                raw, msum = state[m]
                if m == QH + 1:
                    continue
                lnm = lnm_pool.tile([D, ST], BF16, name="lnm")
                nc.scalar.activation(
                    lnm[:], msum[:], mybir.ActivationFunctionType.Ln,
                    bias=eps_sb[:], scale=1.0 / D,
                )
                state[m] = (raw, lnm)
            return (c, blocks, state)

        def epilogue_s1(c, blocks, qkv_ps):
            return epilogue_ln(epilogue_msums(epilogue_drains(c, blocks, qkv_ps)))

        def epilogue_s2(c, blocks, state):
            ssl = bass.ts(c, ST)
            cosq_t, sinq_t, cosk_t, sink_t = tbl_tiles[c]
            for m in blocks:
                raw, lnm = state[m]
                if m == QH + 1:
                    # v block: 4x f32 PE transposes [128,128] into one scratch
                    # PSUM tile, then a single [D, ST] copy into v_sb
                    tp = scr_tile()
                    for cc in range(RB):
                        nc.tensor.transpose(
                            tp[:, bass.ts(cc, D)], raw[:, bass.ts(cc, D)],
                            identity[:],
                        )
                    nc.vector.tensor_copy(
                        v_sb[:, c * RB : (c + 1) * RB, :], tp[:]
                    )
                    continue
                cos_t, sin_t = (cosq_t, sinq_t) if m < QH else (cosk_t, sink_t)
                # rstd = exp(-0.5 * ln(mean + eps)), broadcast on partitions
                rstd = tmp_pool.tile([D, ST], F32, name="rstd")
                nc.scalar.activation(
                    rstd[:], lnm[:], mybir.ActivationFunctionType.Exp,
                    scale=-0.5,
                )
                # rope: rot = raw*cos + swap(raw)*sin_signed (gains in
                # tables; sin table ships half-swapped so both DVE inputs
                # share a partition base - only the output is relocated)
                t1 = tmp_pool.tile([D, ST], F32, name="t1")
                nc.vector.tensor_mul(t1[:], raw[:], cos_t[:])
                t2 = tmp_pool.tile([D, ST], F32, name="t2")
                nc.vector.tensor_mul(t2[:HALF], raw[HALF:], sin_t[HALF:])
                nc.vector.tensor_mul(t2[HALF:], raw[:HALF], sin_t[:HALF])
                nc.vector.tensor_add(t1[:], t1[:], t2[:])
                nc.vector.tensor_mul(qkT[m][:, ssl], t1[:], rstd[:])

        def epilogue(c, blocks, qkv_ps):
            epilogue_s2(*epilogue_s1(c, blocks, qkv_ps))

        def attn_head(a, h):
            # attention for head h on query chunk a; diagonal-band blocks
            # (r >= 0) restrict every matmul + the exp to columns >= r*128
            ssl = bass.ts(a, ST)
            n_sk = (a + 1) * RB
            ctx_ps = cd_ps_pool.tile([D, ST], F32, name="ctx_ps")
            den_ps = cd_ps_pool.tile([D, ST], F32, name="den_ps")
            for ski in range(n_sk):
                r = ski - a * RB
                lo = max(r, 0) * D  # first valid column in this chunk
                sc = scr_tile()
                nc.tensor.matmul(
                    sc[:, lo:],
                    qkT[QH][:, bass.ts(ski, D)],
                    qkT[h][:, a * ST + lo : (a + 1) * ST],
                    start=True,
                    stop=True,
                )
                e_sb = e_pool.tile([D, ST], BF16, name="e_sb")
                nc.scalar.activation(
                    e_sb[:, lo:], sc[:, lo:],
                    mybir.ActivationFunctionType.Exp,
                    scale=SCALE,
                )
                if r >= 0:
                    # triangle mask on the [128,128] diagonal sub-block
                    nc.vector.tensor_mul(
                        e_sb[:, lo : lo + D], e_sb[:, lo : lo + D], mask_sb[:]
                    )
                first = ski == 0
                last = ski == n_sk - 1
                nc.tensor.matmul(
                    ctx_ps[:, lo:], v_sb[:, ski, :], e_sb[:, lo:],
                    start=first, stop=last,
                )
                nc.tensor.matmul(
                    den_ps[:, lo:], ones_sb[:], e_sb[:, lo:],
                    start=first, stop=last,
                )
            recip = a_tmp.tile([D, ST], F32, name="recip")
            nc.vector.reciprocal_approx_fast(recip[:], den_ps[:])
            nc.vector.tensor_mul(ctx_sb[h][:, ssl], ctx_ps[:], recip[:])

        def oproj_sti(o, sti):
            # o_proj for row-tile sti of chunk o
            st = o * RB + sti
            for ntg in range(2):
                # 4 nt-tiles drain into one [D, 4, ST] tile -> ONE out DMA
                # (each DMA costs ~650ns of gpsimd queue issue time; 128
                # separate ones would throttle the o_proj tail)
                out_sb = o_pool.tile([D, 4, ST], BF16, name="out_sb")
                for nti in range(4):
                    nt = ntg * 4 + nti
                    ps = scr_tile()
                    for ht in range(QH):
                        nc.tensor.matmul(
                            ps[:],
                            ctx_sb[ht][:, bass.ts(st, D)],
                            wo_sb[:, ht, bass.ts(nt, ST)],
                            start=(ht == 0),
                            stop=(ht == QH - 1),
                        )
                    # alternate the PSUM drains between DVE and ACT so
                    # neither FIFO gates the scr bank rotation
                    if nt % 2 == 0:
                        nc.vector.tensor_copy(out_sb[:, nti, :], ps[:])
                    else:
                        nc.scalar.copy(out_sb[:, nti, :], ps[:])
                nc.gpsimd.dma_start(
                    out_d[bass.ts(st, D), bass.ts(ntg, 4 * ST)], out_sb[:]
                )

        # ================= fused pipeline =================
        load_tables(0)
        for step in range(N_ST + 2):
            c = step  # qkv chunk
            a = step - 1  # attention chunk
            o = step - 2  # o_proj chunk
            eplg_todo = []
            if c < N_ST:
                if c + 1 < N_ST:
                    load_tables(c + 1)
                # wave/epilogue interleave (waves: q01, k, q23, v). Each
                # wave's msum matmuls land on the PE queue only after its
                # drains had a full wave of cover. ACT order per chunk:
                # [drains, ln x5 (one nat-log table load), rstd-exps (one
                # exp table load, shared with attention)] -- the last lns
                # gate on the mid-v msums and everything completes under
                # the v wave; q2/q3's rstd/rope lag past attn h0 harmlessly
                # (same table set as the attention exps).
                ps_01 = qkv_wave(c, WAVES[0], c == 0)
                ps_k = qkv_wave(c, WAVES[1], False)
                s1_01 = epilogue_s1(c, WAVES[0], ps_01)
                ps_23 = qkv_wave(c, WAVES[2], False)
                if c + 1 < N_ST:
                    load_hid(c + 1)
                s1_k = epilogue_s1(c, WAVES[1], ps_k)
                dr_23 = epilogue_drains(c, WAVES[2], ps_23)
                holder = {}
                ps_v = qkv_wave(
                    c, WAVES[3], False,
                    mid_kt=10,
                    mid_cb=lambda: holder.update(
                        s=epilogue_ln(epilogue_msums(dr_23))
                    ),
                )
                if c == 0:
                    # w_o prefetch on the sync queue AFTER chunk 0/1's loads:
                    # the in-order queue delays the (big) transfer past the
                    # startup window so it can't starve the first matmuls
                    nc.sync.dma_start(
                        wo_sb[:], wo_d.rearrange("(h p) n -> p h n", p=D)
                    )
                epilogue_s2(*s1_k)
                epilogue_s2(*s1_01)
                s23 = holder["s"]
                eplg_todo.append(lambda s23=s23, c=c, ps_v=ps_v: (
                    epilogue_s2(*s23),
                    epilogue_s2(*epilogue_drains(c, WAVES[3], ps_v)),
                ))

            if 0 <= a <= N_ST - 1:
                for h in range(QH):
                    attn_head(a, h)
                    if eplg_todo and h == 0:
                        eplg_todo.pop()()
                    if o >= 0:
                        oproj_sti(o, h)
            else:
                if eplg_todo:
                    eplg_todo.pop()()
                if 0 <= o:
                    for sti in range(RB):
                        oproj_sti(o, sti)


def _host_prep(positions, hidden_states, w_qkv, w_o, gq, gk):
    import ml_dtypes

    bf = ml_dtypes.bfloat16

    positions = np.asarray(positions)
    hidden_states = np.asarray(hidden_states, dtype=np.float32)
    w_qkv = np.asarray(w_qkv, dtype=np.float32)
    w_o = np.asarray(w_o, dtype=np.float32)
    gq = np.asarray(gq, dtype=np.float32)
    gk = np.asarray(gk, dtype=np.float32)

    hidden_t = np.ascontiguousarray(hidden_states.T).astype(bf)  # [HID, S]

    inv_freq = 1.0 / (THETA ** (np.arange(HALF, dtype=np.float32) * 2.0 / D))
    freqs = positions.astype(np.float32)[:, None] * inv_freq  # [S, HALF]
    cos = np.cos(freqs)
    sin = np.sin(freqs)

    def rope_tables(g):
        # cos_t[d, s] = cos[s, d % HALF] * g[d]
        cos_t = np.concatenate([cos * g[None, :HALF], cos * g[None, HALF:]], axis=1).T
        # rot[d] = x[d] cos[d] - x[d+HALF] sin[d]   (d < HALF)
        #          x[d] cos[d] + x[d-HALF] sin[d]   (d >= HALF)
        # The partner's gain is folded into the sin table, and the table ships
        # HALF-SWAPPED: row j holds the sin factor that multiplies x[j], i.e.
        # the factor for output row j^HALF. This keeps both DVE inputs of the
        # rope cross-multiply at the same partition base.
        sin_t = np.concatenate([sin * g[None, :HALF], -sin * g[None, HALF:]], axis=1).T
        return (
            np.ascontiguousarray(cos_t, dtype=np.float16),
            np.ascontiguousarray(sin_t, dtype=np.float16),
        )

    cosq, sinq = rope_tables(gq)
    cosk, sink = rope_tables(gk)

    # lower-triangle [128,128] mask for the diagonal sub-blocks
    k_idx = np.arange(D)
    mask_tri = (k_idx[:, None] <= k_idx[None, :]).astype(bf)

    per_core = []
    for c in range(NCORES):
        q_rows = w_qkv[c * QH * D : (c + 1) * QH * D]  # [512, HID]
        k_rows = w_qkv[H * D + c * D : H * D + (c + 1) * D]  # [128, HID]
        v_rows = w_qkv[(H + KV) * D + c * D : (H + KV) * D + (c + 1) * D]
        wqkv_c = np.concatenate([q_rows, k_rows, v_rows], axis=0)  # [768, HID]
        wqkv_t = np.ascontiguousarray(wqkv_c.T).astype(bf)  # [HID, 768]
        wo_t = np.ascontiguousarray(
            w_o[:, c * QH * D : (c + 1) * QH * D].T
        ).astype(bf)
        per_core.append(
            {
                "hidden_t": hidden_t,
                "w_qkvT": wqkv_t,
                "w_oT": wo_t,
                "cosq": cosq,
                "sinq": sinq,
                "cosk": cosk,
                "sink": sink,
                "mask_tri": mask_tri,
            }
        )
    return per_core


_NC_CACHE = {}


def _get_nc():
    if "nc" not in _NC_CACHE:
        _NC_CACHE["nc"] = build_bass()
    return _NC_CACHE["nc"]


def kernel(positions, hidden_states, w_qkv, w_o, gq, gk, _trace=False):
    in_maps = _host_prep(positions, hidden_states, w_qkv, w_o, gq, gk)
    nc = _get_nc()
    res = run_bass_kernel_spmd(
        nc, in_maps, core_ids=list(range(NCORES)), trace=_trace
    )
    out = np.zeros((S, HID), dtype=np.float32)
    for r in res.results:
        out += r["out_partial"].astype(np.float32)
    if _trace:
        kernel._last_results = res
    return out


# revision 45
# speedup vs baseline: 1.0270x; 1.0270x over previous
"""Trainium2 Bass kernel for a GQA attention layer (S=2048, HID=4096, H=32, KV=8, D=128).

Sharding: tensor-parallel over heads across 8 NeuronCores. Core c computes
q heads [4c, 4c+4) and kv head c end-to-end (QKV proj -> RMSNorm -> RoPE ->
causal flash-style attention -> partial o_proj). Each core returns a partial
[S, HID] bf16 o_proj output (w_o column-sharded); the host sums the 8 partials.

Device layout notes:
- All projections run with the feature dim on PSUM partitions: qkv_out[f, s] =
  (w_qkvT tile).T @ hidden_T tile, so q/k arrive as [D, S] (head dim on
  partitions), which is exactly the layout the scores matmul needs
  (contraction over D).
- Scores are computed transposed: scoresT[k, q] via lhsT=kT tile [D, Sk-128],
  rhs=qT [D, Sq-512]. The softmax denominator is ones[128,128].T @ exp(scoresT),
  which also broadcasts the per-q sum across all 128 partitions so the
  normalization multiply needs no cross-partition traffic. No max-subtraction:
  scores are O(5) for RMS-normed q/k, so exp is safe in fp32.
- Causal masking at 128-column granularity: blocks above the diagonal are
  skipped outright; the 4 diagonal-band blocks per (head, sq-chunk) run
  partial-free-dim matmuls (scores AND ctx/den accumulation restricted to
  columns >= r*128) plus one [128,128] triangle mask multiply after exp.
  Zero-column overhead vs the causal ideal at this granularity.
- Single fused pipeline: chunk c's QKV matmuls, chunk c-1's attention and
  chunk c-2's o_proj are emitted interleaved, so the PE queue never drains
  across phase boundaries (HAM stays warm) and ACT/DVE epilogues hide under
  matmuls of neighboring stages.
- Matmul operands are bf16 (fp32 PSUM accumulation); the rmsnorm/rope/softmax
  normalization chain stays fp32. fp8 was measured and rejected: e4m3's ~2.6%
  per-element quantization noise does NOT average down in dot products, giving
  2.3-6.8e-2 output rel-err per fp8 stage (vs the 2e-2 budget).
"""

import numpy as np

import concourse.bass as bass
from concourse import bacc
import concourse.tile as tile
from concourse import mybir
from concourse.bass_utils import run_bass_kernel_spmd
from concourse.masks import make_identity

F32 = mybir.dt.float32
F16 = mybir.dt.float16
BF16 = mybir.dt.bfloat16

S = 2048
HID = 4096
H = 32
KV = 8
D = 128
QH = H // KV  # 4 q heads per kv head; with 8 cores -> 1 kv head per core
NCORES = 8
EPS = 1e-6
THETA = 10000.0
HALF = D // 2

ST = 512  # seq chunk (matmul free dim)
N_ST = S // ST  # 4
KT = HID // D  # 32 contraction tiles for qkv
NB = QH + 2  # 6 feature blocks per core: q0..q3, k, v
SK = S // D  # 16 key blocks of 128
SCALE = float(D) ** -0.5
RB = ST // D  # 4 row-tiles / diagonal bands per chunk

# feature-block waves per chunk, in emission order: v last. The k and v
# blocks get their own waves so every ACT Sqrt of a chunk (q0..q3 + k
# rmsnorm) is emitted before the attention exps; only the v epilogue
# (transposes, no Sqrt) lands after attention starts. A tiny dummy exp
# after the sqrts pulls the exp-table reload under the v wave's matmuls.
# (A batched ln+exp rsqrt was tried instead: the Tile scheduler reorders
# the ACT stream by readiness, un-batching it into 25 table loads.)
WAVES = [(0, 1), (2, 3), (4,), (5,)]  # 4 = k, 5 = v


def build_bass():
    nc = bacc.Bacc(
        "TRN2", target_bir_lowering=False, debug=False, num_devices=NCORES
    )
    hid_d = nc.dram_tensor("hidden_t", [HID, S], BF16, kind="ExternalInput").ap()
    wqkv_d = nc.dram_tensor("w_qkvT", [HID, NB * D], BF16, kind="ExternalInput").ap()
    wo_d = nc.dram_tensor("w_oT", [QH * D, HID], BF16, kind="ExternalInput").ap()
    cosq_d = nc.dram_tensor("cosq", [D, S], F16, kind="ExternalInput").ap()
    sinq_d = nc.dram_tensor("sinq", [D, S], F16, kind="ExternalInput").ap()
    cosk_d = nc.dram_tensor("cosk", [D, S], F16, kind="ExternalInput").ap()
    sink_d = nc.dram_tensor("sink", [D, S], F16, kind="ExternalInput").ap()
    mask_d = nc.dram_tensor("mask_tri", [D, D], BF16, kind="ExternalInput").ap()
    out_d = nc.dram_tensor("out_partial", [S, HID], BF16, kind="ExternalOutput").ap()

    with tile.TileContext(nc) as tc:
        build_kernel(
            nc, tc, hid_d, wqkv_d, wo_d,
            (cosq_d, sinq_d, cosk_d, sink_d), mask_d, out_d,
        )
    nc.finalize()
    return nc


def build_kernel(nc, tc, hid_d, wqkv_d, wo_d, tables_d, mask_d, out_d):
    from contextlib import ExitStack

    cosq_d, sinq_d, cosk_d, sink_d = tables_d

    with ExitStack() as ctxs:
        # ---- persistent SBUF ----
        qk_pool = ctxs.enter_context(tc.tile_pool(name="qk", bufs=1))
        const_pool = ctxs.enter_context(tc.tile_pool(name="const", bufs=1))
        w_pool = ctxs.enter_context(tc.tile_pool(name="wqkv", bufs=1))
        wo_pool = ctxs.enter_context(tc.tile_pool(name="wo", bufs=1))
        ctx_pool = ctxs.enter_context(tc.tile_pool(name="ctx", bufs=1))
        hid_pool = ctxs.enter_context(tc.tile_pool(name="hid", bufs=9))
        tbl_pool = ctxs.enter_context(tc.tile_pool(name="tbl", bufs=2))
        tmp_pool = ctxs.enter_context(tc.tile_pool(name="p1tmp", bufs=1))
        drain_pool = ctxs.enter_context(tc.tile_pool(name="drain", bufs=4))
        e_pool = ctxs.enter_context(tc.tile_pool(name="expp", bufs=6))
        a_tmp = ctxs.enter_context(tc.tile_pool(name="p2tmp", bufs=3))
        o_pool = ctxs.enter_context(tc.tile_pool(name="outst", bufs=3))
        # PSUM budget (bufs is per tile NAME): qkv wave slots lo/hi x2 = 4
        # banks; one shared scratch name (msum/v-transpose/scores/o_proj)
        # x2 = 2; ctx + den x1 each = 2  -> 8 banks total
        qkv_ps_pool = ctxs.enter_context(
            tc.tile_pool(name="qkv_ps", bufs=2, space="PSUM")
        )
        scr_ps_pool = ctxs.enter_context(
            tc.tile_pool(name="scr_ps", bufs=2, space="PSUM")
        )
        cd_ps_pool = ctxs.enter_context(
            tc.tile_pool(name="cd_ps", bufs=1, space="PSUM")
        )

        def scr_tile():
            return scr_ps_pool.tile([D, ST], F32, name="scr_ps")

        qkT = [qk_pool.tile([D, S], BF16, name=f"qkT_{m}") for m in range(QH + 1)]
        v_sb = qk_pool.tile([D, SK, D], BF16, name="v_sb")
        ctx_sb = [qk_pool.tile([D, S], BF16, name=f"ctx_{h}") for h in range(QH)]
        wqkv_sb = w_pool.tile([D, KT, NB * D], BF16, name="wqkv_sb")
        wo_sb = wo_pool.tile([D, QH, HID], BF16, name="wo_sb")
        mask_sb = const_pool.tile([D, D], BF16, name="mask_sb")

        identity = const_pool.tile([D, D], F32, name="identity")
        make_identity(nc, identity[:])
        ones_f32 = const_pool.tile([D, D], F32, name="ones_f32")
        nc.vector.memset(ones_f32[:], 1.0)
        ones_sb = const_pool.tile([D, D], BF16, name="ones_sb")
        nc.vector.tensor_copy(ones_sb[:], ones_f32[:])
        eps_sb = const_pool.tile([D, 1], F32, name="eps_sb")
        nc.vector.memset(eps_sb[:], EPS)

        # triangle mask on the gpsimd DMA queue (the sync queue carries the
        # streaming hidden/table loads). The (big) w_o prefetch is emitted
        # after chunk 0's first wave so it can't delay the wqkv slices the
        # very first matmul needs.
        nc.gpsimd.dma_start(mask_sb[:], mask_d)

        hid_tiles = [[None] * KT for _ in range(N_ST)]
        tbl_tiles = [None] * N_ST

        def load_tables(c):
            # rope tables ride the (idle) gpsimd DMA queue, off the sync
            # queue's critical startup path
            tbls = []
            for i, td in enumerate((cosq_d, sinq_d, cosk_d, sink_d)):
                t = tbl_pool.tile([D, ST], F16, name=f"tbl_{i}")
                nc.gpsimd.dma_start(t[:], td[:, bass.ts(c, ST)])
                tbls.append(t)
            tbl_tiles[c] = tbls

        # hidden loads are batched 4 kt-tiles per DMA: each DIRECT2D costs
        # ~600ns of issue time on the sync queue regardless of size, and
        # 128 separate loads would choke the queue
        def load_hid_batch(c, tb):
            t = hid_pool.tile([D, 4, ST], BF16, name="hid_t")
            nc.sync.dma_start(
                t[:],
                hid_d[bass.ts(tb, 4 * D), bass.ts(c, ST)].rearrange(
                    "(t p) n -> p t n", p=D
                ),
            )
            for i in range(4):
                hid_tiles[c][4 * tb + i] = t[:, i, :]

        def load_hid(c):
            for tb in range(KT // 4):
                load_hid_batch(c, tb)

        def qkv_wave(c, wave, do_loads, mid_kt=None, mid_cb=None):
            qkv_ps = {
                m: qkv_ps_pool.tile(
                    [D, ST], F32, name=f"qkv_ps_{'lo' if i == 0 else 'hi'}"
                )
                for i, m in enumerate(wave)
            }
            for kt in range(KT):
                if kt == mid_kt:
                    mid_cb()
                if do_loads:
                    # interleave weight + hidden loads on the (fast HWDGE)
                    # sync queue so the PE starts immediately: kt 0..3 as
                    # singles (first matmul waits only two small DMAs), the
                    # rest 4-batched to bound queue-issue overhead
                    if kt < 4:
                        nc.sync.dma_start(
                            wqkv_sb[:, kt, :], wqkv_d[bass.ts(kt, D), :]
                        )
                        if kt == 0:
                            t0 = hid_pool.tile([D, 4, ST], BF16, name="hid_t")
                        nc.sync.dma_start(
                            t0[:, kt, :], hid_d[bass.ts(kt, D), bass.ts(c, ST)]
                        )
                        hid_tiles[c][kt] = t0[:, kt, :]
                    elif kt % 4 == 0:
                        tb = kt // 4
                        nc.sync.dma_start(
                            wqkv_sb[:, bass.ts(tb, 4), :],
                            wqkv_d[bass.ts(tb, 4 * D), :].rearrange(
                                "(t p) n -> p t n", p=D
                            ),
                        )
                        load_hid_batch(c, tb)
                t = hid_tiles[c][kt]
                for m in wave:
                    nc.tensor.matmul(
                        qkv_ps[m][:],
                        wqkv_sb[:, kt, bass.ts(m, D)],
                        t,
                        start=(kt == 0),
                        stop=(kt == KT - 1),
                    )
            return qkv_ps

        def epilogue_drains(c, blocks, qkv_ps):
            # PSUM drains + squared copies (ACT + DVE, no PE)
            state = {}
            for m in blocks:
                raw = drain_pool.tile([D, ST], F32, name="raw")
                nc.scalar.copy(raw[:], qkv_ps[m][:])
                if m == QH + 1:
                    state[m] = (raw, None)
                    continue
                sq = tmp_pool.tile([D, ST], BF16, name="sq")
                nc.vector.tensor_mul(sq[:], raw[:], raw[:])
                state[m] = (raw, sq)
            return (c, blocks, state)

        def epilogue_msums(st8):
            # (PE) mean-square matmuls; emitted where the drains are known
            # to have completed so the PE never stalls on them
            c, blocks, state = st8
            for m in blocks:
                raw, sq = state[m]
                if m == QH + 1:
                    continue
                msum = scr_tile()
                nc.tensor.matmul(msum[:], ones_sb[:], sq[:], start=True, stop=True)
                state[m] = (raw, msum)
            return (c, blocks, state)

        def epilogue_s1(c, blocks, qkv_ps):
            return epilogue_msums(epilogue_drains(c, blocks, qkv_ps))

        def epilogue_s2(c, blocks, state):
            ssl = bass.ts(c, ST)
            cosq_t, sinq_t, cosk_t, sink_t = tbl_tiles[c]
            for m in blocks:
                raw, msum = state[m]
                if m == QH + 1:
                    # v block: 4x f32 PE transposes [128,128] into one scratch
                    # PSUM tile, then a single [D, ST] copy into v_sb
                    tp = scr_tile()
                    for cc in range(RB):
                        nc.tensor.transpose(
                            tp[:, bass.ts(cc, D)], raw[:, bass.ts(cc, D)],
                            identity[:],
                        )
                    nc.vector.tensor_copy(
                        v_sb[:, c * RB : (c + 1) * RB, :], tp[:]
                    )
                    continue
                cos_t, sin_t = (cosq_t, sinq_t) if m < QH else (cosk_t, sink_t)
                # rstd = 1/sqrt(mean + eps), broadcast on partitions
                rstd = tmp_pool.tile([D, ST], F32, name="rstd")
                nc.scalar.activation(
                    rstd[:], msum[:], mybir.ActivationFunctionType.Sqrt,
                    bias=eps_sb[:], scale=1.0 / D,
                )
                nc.vector.reciprocal_approx_fast(rstd[:], rstd[:])
                # rope: rot = raw*cos + swap(raw)*sin_signed (gains in
                # tables; sin table ships half-swapped so both DVE inputs
                # share a partition base - only the output is relocated)
                t1 = tmp_pool.tile([D, ST], F32, name="t1")
                nc.vector.tensor_mul(t1[:], raw[:], cos_t[:])
                t2 = tmp_pool.tile([D, ST], F32, name="t2")
                nc.vector.tensor_mul(t2[:HALF], raw[HALF:], sin_t[HALF:])
                nc.vector.tensor_mul(t2[HALF:], raw[:HALF], sin_t[:HALF])
                nc.vector.tensor_add(t1[:], t1[:], t2[:])
                nc.vector.tensor_mul(qkT[m][:, ssl], t1[:], rstd[:])

        def epilogue(c, blocks, qkv_ps):
            epilogue_s2(*epilogue_s1(c, blocks, qkv_ps))

        def attn_head(a, h):
            # attention for head h on query chunk a; diagonal-band blocks
            # (r >= 0) restrict every matmul + the exp to columns >= r*128
            ssl = bass.ts(a, ST)
            n_sk = (a + 1) * RB
            ctx_ps = cd_ps_pool.tile([D, ST], F32, name="ctx_ps")
            den_ps = cd_ps_pool.tile([D, ST], F32, name="den_ps")
            for ski in range(n_sk):
                r = ski - a * RB
                lo = max(r, 0) * D  # first valid column in this chunk
                sc = scr_tile()
                nc.tensor.matmul(
                    sc[:, lo:],
                    qkT[QH][:, bass.ts(ski, D)],
                    qkT[h][:, a * ST + lo : (a + 1) * ST],
                    start=True,
                    stop=True,
                )
                e_sb = e_pool.tile([D, ST], BF16, name="e_sb")
                nc.scalar.activation(
                    e_sb[:, lo:], sc[:, lo:],
                    mybir.ActivationFunctionType.Exp,
                    scale=SCALE,
                )
                if r >= 0:
                    # triangle mask on the [128,128] diagonal sub-block
                    nc.vector.tensor_mul(
                        e_sb[:, lo : lo + D], e_sb[:, lo : lo + D], mask_sb[:]
                    )
                first = ski == 0
                last = ski == n_sk - 1
                nc.tensor.matmul(
                    ctx_ps[:, lo:], v_sb[:, ski, :], e_sb[:, lo:],
                    start=first, stop=last,
                )
                nc.tensor.matmul(
                    den_ps[:, lo:], ones_sb[:], e_sb[:, lo:],
                    start=first, stop=last,
                )
            recip = a_tmp.tile([D, ST], F32, name="recip")
            nc.vector.reciprocal_approx_fast(recip[:], den_ps[:])
            nc.vector.tensor_mul(ctx_sb[h][:, ssl], ctx_ps[:], recip[:])

        def oproj_sti(o, sti):
            # o_proj for row-tile sti of chunk o
            st = o * RB + sti
            for ntg in range(2):
                # 4 nt-tiles drain into one [D, 4, ST] tile -> ONE out DMA
                # (each DMA costs ~650ns of gpsimd queue issue time; 128
                # separate ones would throttle the o_proj tail)
                out_sb = o_pool.tile([D, 4, ST], BF16, name="out_sb")
                for nti in range(4):
                    nt = ntg * 4 + nti
                    ps = scr_tile()
                    for ht in range(QH):
                        nc.tensor.matmul(
                            ps[:],
                            ctx_sb[ht][:, bass.ts(st, D)],
                            wo_sb[:, ht, bass.ts(nt, ST)],
                            start=(ht == 0),
                            stop=(ht == QH - 1),
                        )
                    # alternate the PSUM drains between DVE and ACT so
                    # neither FIFO gates the scr bank rotation
                    if nt % 2 == 0:
                        nc.vector.tensor_copy(out_sb[:, nti, :], ps[:])
                    else:
                        nc.scalar.copy(out_sb[:, nti, :], ps[:])
                nc.gpsimd.dma_start(
                    out_d[bass.ts(st, D), bass.ts(ntg, 4 * ST)], out_sb[:]
                )

        # ================= fused pipeline =================
        load_tables(0)
        for step in range(N_ST + 2):
            c = step  # qkv chunk
            a = step - 1  # attention chunk
            o = step - 2  # o_proj chunk
            eplg_todo = []
            if c < N_ST:
                if c + 1 < N_ST:
                    load_tables(c + 1)
                # wave/epilogue interleave (waves: q01, q23, k, v): each
                # wave's msum matmuls land on the PE queue only after the
                # next wave's matmuls, so drains have a full wave of cover;
                # all Sqrts + the exp-table load complete under the k/v
                # waves' matmuls
                ps_01 = qkv_wave(c, WAVES[0], c == 0)
                ps_23 = qkv_wave(c, WAVES[1], False)
                s1_01 = epilogue_s1(c, WAVES[0], ps_01)
                ps_k = qkv_wave(c, WAVES[2], False)
                if c + 1 < N_ST:
                    load_hid(c + 1)
                epilogue_s2(*s1_01)
                s1_23 = epilogue_s1(c, WAVES[1], ps_23)
                s1_k = epilogue_s1(c, WAVES[2], ps_k)
                ps_v = qkv_wave(c, WAVES[3], False)
                if c == 0:
                    # w_o prefetch on the sync queue AFTER chunk 0/1's loads:
                    # the in-order queue delays the (big) transfer past the
                    # startup window so it can't starve the first matmuls
                    nc.sync.dma_start(
                        wo_sb[:], wo_d.rearrange("(h p) n -> p h n", p=D)
                    )
                epilogue_s2(*s1_23)
                epilogue_s2(*s1_k)
                eplg_todo.append(lambda c=c, ps_v=ps_v: epilogue_s2(
                    *epilogue_drains(c, WAVES[3], ps_v)
                ))
                # tiny dummy exp: hoists the exp-table load here, where it
                # hides under the v wave's matmuls instead of stalling the
                # first attention block's exp -> ctx chain
                dummy = a_tmp.tile([D, 1], F32, name="dummy_exp")
                nc.scalar.activation(
                    dummy[:], eps_sb[:], mybir.ActivationFunctionType.Exp,
                )

            if 0 <= a <= N_ST - 1:
                for h in range(QH):
                    attn_head(a, h)
                    if eplg_todo and h == 0:
                        eplg_todo.pop()()
                    if o >= 0:
                        oproj_sti(o, h)
            else:
                if eplg_todo:
                    eplg_todo.pop()()
                if 0 <= o:
                    for sti in range(RB):
                        oproj_sti(o, sti)


def _host_prep(positions, hidden_states, w_qkv, w_o, gq, gk):
    import ml_dtypes

    bf = ml_dtypes.bfloat16

    positions = np.asarray(positions)
    hidden_states = np.asarray(hidden_states, dtype=np.float32)
    w_qkv = np.asarray(w_qkv, dtype=np.float32)
    w_o = np.asarray(w_o, dtype=np.float32)
    gq = np.asarray(gq, dtype=np.float32)
    gk = np.asarray(gk, dtype=np.float32)

    hidden_t = np.ascontiguousarray(hidden_states.T).astype(bf)  # [HID, S]

    inv_freq = 1.0 / (THETA ** (np.arange(HALF, dtype=np.float32) * 2.0 / D))
    freqs = positions.astype(np.float32)[:, None] * inv_freq  # [S, HALF]
    cos = np.cos(freqs)
    sin = np.sin(freqs)

    def rope_tables(g):
        # cos_t[d, s] = cos[s, d % HALF] * g[d]
        cos_t = np.concatenate([cos * g[None, :HALF], cos * g[None, HALF:]], axis=1).T
        # rot[d] = x[d] cos[d] - x[d+HALF] sin[d]   (d < HALF)
        #          x[d] cos[d] + x[d-HALF] sin[d]   (d >= HALF)
        # The partner's gain is folded into the sin table, and the table ships
        # HALF-SWAPPED: row j holds the sin factor that multiplies x[j], i.e.
        # the factor for output row j^HALF. This keeps both DVE inputs of the
        # rope cross-multiply at the same partition base.
        sin_t = np.concatenate([sin * g[None, :HALF], -sin * g[None, HALF:]], axis=1).T
        return (
            np.ascontiguousarray(cos_t, dtype=np.float16),
            np.ascontiguousarray(sin_t, dtype=np.float16),
        )

    cosq, sinq = rope_tables(gq)
    cosk, sink = rope_tables(gk)

    # lower-triangle [128,128] mask for the diagonal sub-blocks
    k_idx = np.arange(D)
    mask_tri = (k_idx[:, None] <= k_idx[None, :]).astype(bf)

    per_core = []
    for c in range(NCORES):
        q_rows = w_qkv[c * QH * D : (c + 1) * QH * D]  # [512, HID]
        k_rows = w_qkv[H * D + c * D : H * D + (c + 1) * D]  # [128, HID]
        v_rows = w_qkv[(H + KV) * D + c * D : (H + KV) * D + (c + 1) * D]
        wqkv_c = np.concatenate([q_rows, k_rows, v_rows], axis=0)  # [768, HID]
        wqkv_t = np.ascontiguousarray(wqkv_c.T).astype(bf)  # [HID, 768]
        wo_t = np.ascontiguousarray(
            w_o[:, c * QH * D : (c + 1) * QH * D].T
        ).astype(bf)
        per_core.append(
            {
                "hidden_t": hidden_t,
                "w_qkvT": wqkv_t,
                "w_oT": wo_t,
                "cosq": cosq,
                "sinq": sinq,
                "cosk": cosk,
                "sink": sink,
                "mask_tri": mask_tri,
            }
        )
    return per_core


_NC_CACHE = {}


def _get_nc():
    if "nc" not in _NC_CACHE:
        _NC_CACHE["nc"] = build_bass()
    return _NC_CACHE["nc"]


def kernel(positions, hidden_states, w_qkv, w_o, gq, gk, _trace=False):
    in_maps = _host_prep(positions, hidden_states, w_qkv, w_o, gq, gk)
    nc = _get_nc()
    res = run_bass_kernel_spmd(
        nc, in_maps, core_ids=list(range(NCORES)), trace=_trace
    )
    out = np.zeros((S, HID), dtype=np.float32)
    for r in res.results:
        out += r["out_partial"].astype(np.float32)
    if _trace:
        kernel._last_results = res
    return out


# revision 48
# speedup vs baseline: 1.0318x; 1.0047x over previous
"""Trainium2 Bass kernel for a GQA attention layer (S=2048, HID=4096, H=32, KV=8, D=128).

Sharding: tensor-parallel over heads across 8 NeuronCores. Core c computes
q heads [4c, 4c+4) and kv head c end-to-end (QKV proj -> RMSNorm -> RoPE ->
causal flash-style attention -> partial o_proj). Each core returns a partial
[S, HID] bf16 o_proj output (w_o column-sharded); the host sums the 8 partials.

Device layout notes:
- All projections run with the feature dim on PSUM partitions: qkv_out[f, s] =
  (w_qkvT tile).T @ hidden_T tile, so q/k arrive as [D, S] (head dim on
  partitions), which is exactly the layout the scores matmul needs
  (contraction over D).
- Scores are computed transposed: scoresT[k, q] via lhsT=kT tile [D, Sk-128],
  rhs=qT [D, Sq-512]. The softmax denominator is ones[128,128].T @ exp(scoresT),
  which also broadcasts the per-q sum across all 128 partitions so the
  normalization multiply needs no cross-partition traffic. No max-subtraction:
  scores are O(5) for RMS-normed q/k, so exp is safe in fp32.
- Causal masking at 128-column granularity: blocks above the diagonal are
  skipped outright; the 4 diagonal-band blocks per (head, sq-chunk) run
  partial-free-dim matmuls (scores AND ctx/den accumulation restricted to
  columns >= r*128) plus one [128,128] triangle mask multiply after exp.
  Zero-column overhead vs the causal ideal at this granularity.
- Single fused pipeline: chunk c's QKV matmuls, chunk c-1's attention and
  chunk c-2's o_proj are emitted interleaved, so the PE queue never drains
  across phase boundaries (HAM stays warm) and ACT/DVE epilogues hide under
  matmuls of neighboring stages.
- Matmul operands are bf16 (fp32 PSUM accumulation); the rmsnorm/rope/softmax
  normalization chain stays fp32. fp8 was measured and rejected: e4m3's ~2.6%
  per-element quantization noise does NOT average down in dot products, giving
  2.3-6.8e-2 output rel-err per fp8 stage (vs the 2e-2 budget).
"""

import numpy as np

import concourse.bass as bass
from concourse import bacc
import concourse.tile as tile
from concourse import mybir
from concourse.bass_utils import run_bass_kernel_spmd
from concourse.masks import make_identity

F32 = mybir.dt.float32
F16 = mybir.dt.float16
BF16 = mybir.dt.bfloat16

S = 2048
HID = 4096
H = 32
KV = 8
D = 128
QH = H // KV  # 4 q heads per kv head; with 8 cores -> 1 kv head per core
NCORES = 8
EPS = 1e-6
THETA = 10000.0
HALF = D // 2

ST = 512  # seq chunk (matmul free dim)
N_ST = S // ST  # 4
KT = HID // D  # 32 contraction tiles for qkv
NB = QH + 2  # 6 feature blocks per core: q0..q3, k, v
SK = S // D  # 16 key blocks of 128
SCALE = float(D) ** -0.5
RB = ST // D  # 4 row-tiles / diagonal bands per chunk

# feature-block waves per chunk, in emission order: v last. The k and v
# blocks get their own waves so every ACT Sqrt of a chunk (q0..q3 + k
# rmsnorm) is emitted before the attention exps; only the v epilogue
# (transposes, no Sqrt) lands after attention starts. A tiny dummy exp
# after the sqrts pulls the exp-table reload under the v wave's matmuls.
# (A batched ln+exp rsqrt was tried instead: the Tile scheduler reorders
# the ACT stream by readiness, un-batching it into 25 table loads.)
WAVES = [(0, 1), (2, 3), (4,), (5,)]  # 4 = k, 5 = v


def build_bass():
    nc = bacc.Bacc(
        "TRN2", target_bir_lowering=False, debug=False, num_devices=NCORES
    )
    hid_d = nc.dram_tensor("hidden_t", [HID, S], BF16, kind="ExternalInput").ap()
    wqkv_d = nc.dram_tensor("w_qkvT", [HID, NB * D], BF16, kind="ExternalInput").ap()
    wo_d = nc.dram_tensor("w_oT", [QH * D, HID], BF16, kind="ExternalInput").ap()
    cosq_d = nc.dram_tensor("cosq", [D, S], F16, kind="ExternalInput").ap()
    sinq_d = nc.dram_tensor("sinq", [D, S], F16, kind="ExternalInput").ap()
    cosk_d = nc.dram_tensor("cosk", [D, S], F16, kind="ExternalInput").ap()
    sink_d = nc.dram_tensor("sink", [D, S], F16, kind="ExternalInput").ap()
    mask_d = nc.dram_tensor("mask_tri", [D, D], BF16, kind="ExternalInput").ap()
    out_d = nc.dram_tensor("out_partial", [S, HID], BF16, kind="ExternalOutput").ap()

    with tile.TileContext(nc) as tc:
        build_kernel(
            nc, tc, hid_d, wqkv_d, wo_d,
            (cosq_d, sinq_d, cosk_d, sink_d), mask_d, out_d,
        )
    nc.finalize()
    return nc


def build_kernel(nc, tc, hid_d, wqkv_d, wo_d, tables_d, mask_d, out_d):
    from contextlib import ExitStack

    cosq_d, sinq_d, cosk_d, sink_d = tables_d

    with ExitStack() as ctxs:
        # ---- persistent SBUF ----
        qk_pool = ctxs.enter_context(tc.tile_pool(name="qk", bufs=1))
        const_pool = ctxs.enter_context(tc.tile_pool(name="const", bufs=1))
        w_pool = ctxs.enter_context(tc.tile_pool(name="wqkv", bufs=1))
        wo_pool = ctxs.enter_context(tc.tile_pool(name="wo", bufs=1))
        ctx_pool = ctxs.enter_context(tc.tile_pool(name="ctx", bufs=1))
        hid_pool = ctxs.enter_context(tc.tile_pool(name="hid", bufs=9))
        tbl_pool = ctxs.enter_context(tc.tile_pool(name="tbl", bufs=2))
        tmp_pool = ctxs.enter_context(tc.tile_pool(name="p1tmp", bufs=1))
        drain_pool = ctxs.enter_context(tc.tile_pool(name="drain", bufs=4))
        e_pool = ctxs.enter_context(tc.tile_pool(name="expp", bufs=6))
        a_tmp = ctxs.enter_context(tc.tile_pool(name="p2tmp", bufs=3))
        o_pool = ctxs.enter_context(tc.tile_pool(name="outst", bufs=3))
        # PSUM budget (bufs is per tile NAME): qkv wave slots lo/hi x2 = 4
        # banks; one shared scratch name (msum/v-transpose/scores/o_proj)
        # x2 = 2; ctx + den x1 each = 2  -> 8 banks total
        qkv_ps_pool = ctxs.enter_context(
            tc.tile_pool(name="qkv_ps", bufs=2, space="PSUM")
        )
        scr_ps_pool = ctxs.enter_context(
            tc.tile_pool(name="scr_ps", bufs=2, space="PSUM")
        )
        cd_ps_pool = ctxs.enter_context(
            tc.tile_pool(name="cd_ps", bufs=1, space="PSUM")
        )

        def scr_tile():
            return scr_ps_pool.tile([D, ST], F32, name="scr_ps")

        qkT = [qk_pool.tile([D, S], BF16, name=f"qkT_{m}") for m in range(QH + 1)]
        v_sb = qk_pool.tile([D, SK, D], BF16, name="v_sb")
        ctx_sb = [qk_pool.tile([D, S], BF16, name=f"ctx_{h}") for h in range(QH)]
        wqkv_sb = w_pool.tile([D, KT, NB * D], BF16, name="wqkv_sb")
        wo_sb = wo_pool.tile([D, QH, HID], BF16, name="wo_sb")
        mask_sb = const_pool.tile([D, D], BF16, name="mask_sb")

        identity = const_pool.tile([D, D], F32, name="identity")
        make_identity(nc, identity[:])
        ones_f32 = const_pool.tile([D, D], F32, name="ones_f32")
        nc.vector.memset(ones_f32[:], 1.0)
        ones_sb = const_pool.tile([D, D], BF16, name="ones_sb")
        nc.vector.tensor_copy(ones_sb[:], ones_f32[:])
        eps_sb = const_pool.tile([D, 1], F32, name="eps_sb")
        nc.vector.memset(eps_sb[:], EPS)

        # triangle mask on the gpsimd DMA queue (the sync queue carries the
        # streaming hidden/table loads). The (big) w_o prefetch is emitted
        # after chunk 0's first wave so it can't delay the wqkv slices the
        # very first matmul needs.
        nc.gpsimd.dma_start(mask_sb[:], mask_d)

        hid_tiles = [[None] * KT for _ in range(N_ST)]
        tbl_tiles = [None] * N_ST

        def load_tables(c):
            # rope tables ride the (idle) gpsimd DMA queue, off the sync
            # queue's critical startup path
            tbls = []
            for i, td in enumerate((cosq_d, sinq_d, cosk_d, sink_d)):
                t = tbl_pool.tile([D, ST], F16, name=f"tbl_{i}")
                nc.gpsimd.dma_start(t[:], td[:, bass.ts(c, ST)])
                tbls.append(t)
            tbl_tiles[c] = tbls

        # hidden loads are batched 4 kt-tiles per DMA: each DIRECT2D costs
        # ~600ns of issue time on the sync queue regardless of size, and
        # 128 separate loads would choke the queue
        def load_hid_batch(c, tb):
            t = hid_pool.tile([D, 4, ST], BF16, name="hid_t")
            nc.sync.dma_start(
                t[:],
                hid_d[bass.ts(tb, 4 * D), bass.ts(c, ST)].rearrange(
                    "(t p) n -> p t n", p=D
                ),
            )
            for i in range(4):
                hid_tiles[c][4 * tb + i] = t[:, i, :]

        def load_hid(c):
            for tb in range(KT // 4):
                load_hid_batch(c, tb)

        def qkv_wave(c, wave, do_loads, mid_kt=None, mid_cb=None):
            qkv_ps = {
                m: qkv_ps_pool.tile(
                    [D, ST], F32, name=f"qkv_ps_{'lo' if i == 0 else 'hi'}"
                )
                for i, m in enumerate(wave)
            }
            for kt in range(KT):
                if kt == mid_kt:
                    mid_cb()
                if do_loads:
                    # interleave weight + hidden loads on the (fast HWDGE)
                    # sync queue so the PE starts immediately: kt 0..3 as
                    # singles (first matmul waits only two small DMAs), the
                    # rest 4-batched to bound queue-issue overhead
                    if kt < 4:
                        nc.sync.dma_start(
                            wqkv_sb[:, kt, :], wqkv_d[bass.ts(kt, D), :]
                        )
                        if kt == 0:
                            t0 = hid_pool.tile([D, 4, ST], BF16, name="hid_t")
                        nc.sync.dma_start(
                            t0[:, kt, :], hid_d[bass.ts(kt, D), bass.ts(c, ST)]
                        )
                        hid_tiles[c][kt] = t0[:, kt, :]
                    elif kt % 4 == 0:
                        tb = kt // 4
                        nc.sync.dma_start(
                            wqkv_sb[:, bass.ts(tb, 4), :],
                            wqkv_d[bass.ts(tb, 4 * D), :].rearrange(
                                "(t p) n -> p t n", p=D
                            ),
                        )
                        load_hid_batch(c, tb)
                t = hid_tiles[c][kt]
                for m in wave:
                    nc.tensor.matmul(
                        qkv_ps[m][:],
                        wqkv_sb[:, kt, bass.ts(m, D)],
                        t,
                        start=(kt == 0),
                        stop=(kt == KT - 1),
                    )
            return qkv_ps

        def epilogue_drains(c, blocks, qkv_ps):
            # PSUM drains + squared copies (ACT + DVE, no PE)
            state = {}
            for m in blocks:
                raw = drain_pool.tile([D, ST], F32, name="raw")
                nc.scalar.copy(raw[:], qkv_ps[m][:])
                if m == QH + 1:
                    state[m] = (raw, None)
                    continue
                sq = tmp_pool.tile([D, ST], BF16, name="sq")
                nc.vector.tensor_mul(sq[:], raw[:], raw[:])
                state[m] = (raw, sq)
            return (c, blocks, state)

        def epilogue_msums(st8):
            # (PE) mean-square matmuls; emitted where the drains are known
            # to have completed so the PE never stalls on them
            c, blocks, state = st8
            for m in blocks:
                raw, sq = state[m]
                if m == QH + 1:
                    continue
                msum = scr_tile()
                nc.tensor.matmul(msum[:], ones_sb[:], sq[:], start=True, stop=True)
                state[m] = (raw, msum)
            return (c, blocks, state)

        def epilogue_s1(c, blocks, qkv_ps):
            return epilogue_msums(epilogue_drains(c, blocks, qkv_ps))

        def epilogue_s2(c, blocks, state):
            ssl = bass.ts(c, ST)
            cosq_t, sinq_t, cosk_t, sink_t = tbl_tiles[c]
            last_rstd = None
            for m in blocks:
                raw, msum = state[m]
                if m == QH + 1:
                    # v block: 4x f32 PE transposes [128,128] into one scratch
                    # PSUM tile, then a single [D, ST] copy into v_sb
                    tp = scr_tile()
                    for cc in range(RB):
                        nc.tensor.transpose(
                            tp[:, bass.ts(cc, D)], raw[:, bass.ts(cc, D)],
                            identity[:],
                        )
                    nc.vector.tensor_copy(
                        v_sb[:, c * RB : (c + 1) * RB, :], tp[:]
                    )
                    continue
                cos_t, sin_t = (cosq_t, sinq_t) if m < QH else (cosk_t, sink_t)
                # rstd = 1/sqrt(mean + eps), broadcast on partitions
                rstd = tmp_pool.tile([D, ST], F32, name="rstd")
                nc.scalar.activation(
                    rstd[:], msum[:], mybir.ActivationFunctionType.Sqrt,
                    bias=eps_sb[:], scale=1.0 / D,
                )
                nc.vector.reciprocal_approx_fast(rstd[:], rstd[:])
                # rope: rot = raw*cos + swap(raw)*sin_signed (gains in
                # tables; sin table ships half-swapped so both DVE inputs
                # share a partition base - only the output is relocated)
                t1 = tmp_pool.tile([D, ST], F32, name="t1")
                nc.vector.tensor_mul(t1[:], raw[:], cos_t[:])
                t2 = tmp_pool.tile([D, ST], F32, name="t2")
                nc.vector.tensor_mul(t2[:HALF], raw[HALF:], sin_t[HALF:])
                nc.vector.tensor_mul(t2[HALF:], raw[:HALF], sin_t[:HALF])
                nc.vector.tensor_add(t1[:], t1[:], t2[:])
                nc.vector.tensor_mul(qkT[m][:, ssl], t1[:], rstd[:])
                last_rstd = rstd
            return last_rstd

        def epilogue(c, blocks, qkv_ps):
            epilogue_s2(*epilogue_s1(c, blocks, qkv_ps))

        def attn_head(a, h):
            # attention for head h on query chunk a; diagonal-band blocks
            # (r >= 0) restrict every matmul + the exp to columns >= r*128
            ssl = bass.ts(a, ST)
            n_sk = (a + 1) * RB
            ctx_ps = cd_ps_pool.tile([D, ST], F32, name="ctx_ps")
            den_ps = cd_ps_pool.tile([D, ST], F32, name="den_ps")
            for ski in range(n_sk):
                r = ski - a * RB
                lo = max(r, 0) * D  # first valid column in this chunk
                sc = scr_tile()
                nc.tensor.matmul(
                    sc[:, lo:],
                    qkT[QH][:, bass.ts(ski, D)],
                    qkT[h][:, a * ST + lo : (a + 1) * ST],
                    start=True,
                    stop=True,
                )
                e_sb = e_pool.tile([D, ST], BF16, name="e_sb")
                nc.scalar.activation(
                    e_sb[:, lo:], sc[:, lo:],
                    mybir.ActivationFunctionType.Exp,
                    scale=SCALE,
                )
                if r >= 0:
                    # triangle mask on the [128,128] diagonal sub-block
                    nc.vector.tensor_mul(
                        e_sb[:, lo : lo + D], e_sb[:, lo : lo + D], mask_sb[:]
                    )
                first = ski == 0
                last = ski == n_sk - 1
                nc.tensor.matmul(
                    ctx_ps[:, lo:], v_sb[:, ski, :], e_sb[:, lo:],
                    start=first, stop=last,
                )
                nc.tensor.matmul(
                    den_ps[:, lo:], ones_sb[:], e_sb[:, lo:],
                    start=first, stop=last,
                )
            recip = a_tmp.tile([D, ST], F32, name="recip")
            nc.vector.reciprocal_approx_fast(recip[:], den_ps[:])
            nc.vector.tensor_mul(ctx_sb[h][:, ssl], ctx_ps[:], recip[:])

        def oproj_sti(o, sti):
            # o_proj for row-tile sti of chunk o
            st = o * RB + sti
            for ntg in range(2):
                # 4 nt-tiles drain into one [D, 4, ST] tile -> ONE out DMA
                # (each DMA costs ~650ns of gpsimd queue issue time; 128
                # separate ones would throttle the o_proj tail)
                out_sb = o_pool.tile([D, 4, ST], BF16, name="out_sb")
                for nti in range(4):
                    nt = ntg * 4 + nti
                    ps = scr_tile()
                    for ht in range(QH):
                        nc.tensor.matmul(
                            ps[:],
                            ctx_sb[ht][:, bass.ts(st, D)],
                            wo_sb[:, ht, bass.ts(nt, ST)],
                            start=(ht == 0),
                            stop=(ht == QH - 1),
                        )
                    # alternate the PSUM drains between DVE and ACT so
                    # neither FIFO gates the scr bank rotation
                    if nt % 2 == 0:
                        nc.vector.tensor_copy(out_sb[:, nti, :], ps[:])
                    else:
                        nc.scalar.copy(out_sb[:, nti, :], ps[:])
                nc.gpsimd.dma_start(
                    out_d[bass.ts(st, D), bass.ts(ntg, 4 * ST)], out_sb[:]
                )

        # ================= fused pipeline =================
        load_tables(0)
        for step in range(N_ST + 2):
            c = step  # qkv chunk
            a = step - 1  # attention chunk
            o = step - 2  # o_proj chunk
            eplg_todo = []
            if c < N_ST:
                if c + 1 < N_ST:
                    load_tables(c + 1)
                # wave/epilogue interleave (waves: q01, q23, k, v): each
                # wave's msum matmuls land on the PE queue only after the
                # next wave's matmuls, so drains have a full wave of cover;
                # all Sqrts + the exp-table load complete under the k/v
                # waves' matmuls
                ps_01 = qkv_wave(c, WAVES[0], c == 0)
                ps_23 = qkv_wave(c, WAVES[1], False)
                s1_01 = epilogue_s1(c, WAVES[0], ps_01)
                ps_k = qkv_wave(c, WAVES[2], False)
                if c + 1 < N_ST:
                    load_hid(c + 1)
                epilogue_s2(*s1_01)
                s1_23 = epilogue_s1(c, WAVES[1], ps_23)
                s1_k = epilogue_s1(c, WAVES[2], ps_k)
                ps_v = qkv_wave(c, WAVES[3], False)
                if c == 0:
                    # w_o prefetch on the sync queue AFTER chunk 0/1's loads:
                    # the in-order queue delays the (big) transfer past the
                    # startup window so it can't starve the first matmuls
                    nc.sync.dma_start(
                        wo_sb[:], wo_d.rearrange("(h p) n -> p h n", p=D)
                    )
                epilogue_s2(*s1_23)
                rstd_k = epilogue_s2(*s1_k)
                eplg_todo.append(lambda c=c, ps_v=ps_v: epilogue_s2(
                    *epilogue_drains(c, WAVES[3], ps_v)
                ))
                # tiny dummy exp reading the k-block's rstd: the dependency
                # forces the (readiness-ordered) scheduler to place it -- and
                # thus the exp-table reload -- right AFTER the chunk's last
                # Sqrt, under the v wave's matmuls, instead of hoisting it
                # early and paying the reload at the first attention exp
                dummy = a_tmp.tile([D, 1], F32, name="dummy_exp")
                nc.scalar.activation(
                    dummy[:], rstd_k[:, :1],
                    mybir.ActivationFunctionType.Exp,
                )

            if 0 <= a <= N_ST - 1:
                for h in range(QH):
                    attn_head(a, h)
                    if eplg_todo and h == 0:
                        eplg_todo.pop()()
                    if o >= 0:
                        oproj_sti(o, h)
            else:
                if eplg_todo:
                    eplg_todo.pop()()
                if 0 <= o:
                    for sti in range(RB):
                        oproj_sti(o, sti)


def _host_prep(positions, hidden_states, w_qkv, w_o, gq, gk):
    import ml_dtypes

    bf = ml_dtypes.bfloat16

    positions = np.asarray(positions)
    hidden_states = np.asarray(hidden_states, dtype=np.float32)
    w_qkv = np.asarray(w_qkv, dtype=np.float32)
    w_o = np.asarray(w_o, dtype=np.float32)
    gq = np.asarray(gq, dtype=np.float32)
    gk = np.asarray(gk, dtype=np.float32)

    hidden_t = np.ascontiguousarray(hidden_states.T).astype(bf)  # [HID, S]

    inv_freq = 1.0 / (THETA ** (np.arange(HALF, dtype=np.float32) * 2.0 / D))
    freqs = positions.astype(np.float32)[:, None] * inv_freq  # [S, HALF]
    cos = np.cos(freqs)
    sin = np.sin(freqs)

    def rope_tables(g):
        # cos_t[d, s] = cos[s, d % HALF] * g[d]
        cos_t = np.concatenate([cos * g[None, :HALF], cos * g[None, HALF:]], axis=1).T
        # rot[d] = x[d] cos[d] - x[d+HALF] sin[d]   (d < HALF)
        #          x[d] cos[d] + x[d-HALF] sin[d]   (d >= HALF)
        # The partner's gain is folded into the sin table, and the table ships
        # HALF-SWAPPED: row j holds the sin factor that multiplies x[j], i.e.
        # the factor for output row j^HALF. This keeps both DVE inputs of the
        # rope cross-multiply at the same partition base.
        sin_t = np.concatenate([sin * g[None, :HALF], -sin * g[None, HALF:]], axis=1).T
        return (
            np.ascontiguousarray(cos_t, dtype=np.float16),
            np.ascontiguousarray(sin_t, dtype=np.float16),
        )

    cosq, sinq = rope_tables(gq)
    cosk, sink = rope_tables(gk)

    # lower-triangle [128,128] mask for the diagonal sub-blocks
    k_idx = np.arange(D)
    mask_tri = (k_idx[:, None] <= k_idx[None, :]).astype(bf)

    per_core = []
    for c in range(NCORES):
        q_rows = w_qkv[c * QH * D : (c + 1) * QH * D]  # [512, HID]
        k_rows = w_qkv[H * D + c * D : H * D + (c + 1) * D]  # [128, HID]
        v_rows = w_qkv[(H + KV) * D + c * D : (H + KV) * D + (c + 1) * D]
        wqkv_c = np.concatenate([q_rows, k_rows, v_rows], axis=0)  # [768, HID]
        wqkv_t = np.ascontiguousarray(wqkv_c.T).astype(bf)  # [HID, 768]
        wo_t = np.ascontiguousarray(
            w_o[:, c * QH * D : (c + 1) * QH * D].T
        ).astype(bf)
        per_core.append(
            {
                "hidden_t": hidden_t,
                "w_qkvT": wqkv_t,
                "w_oT": wo_t,
                "cosq": cosq,
                "sinq": sinq,
                "cosk": cosk,
                "sink": sink,
                "mask_tri": mask_tri,
            }
        )
    return per_core


_NC_CACHE = {}


def _get_nc():
    if "nc" not in _NC_CACHE:
        _NC_CACHE["nc"] = build_bass()
    return _NC_CACHE["nc"]


def kernel(positions, hidden_states, w_qkv, w_o, gq, gk, _trace=False):
    in_maps = _host_prep(positions, hidden_states, w_qkv, w_o, gq, gk)
    nc = _get_nc()
    res = run_bass_kernel_spmd(
        nc, in_maps, core_ids=list(range(NCORES)), trace=_trace
    )
    out = np.zeros((S, HID), dtype=np.float32)
    for r in res.results:
        out += r["out_partial"].astype(np.float32)
    if _trace:
        kernel._last_results = res
    return out
